# revision 1
# baseline (speedup 1.0000x reference)
"""CubicPchipKANLayer Trainium2 kernel.

Math: out[b,o] = sum_i PCHIP_interp(x[b,i]; knots y[i,:,o]) + bias[o]

Reformulation: with t = clip((x - D_MIN)/H, 0, K-1), the PCHIP interpolant is
linear over the knot tables:
    out[b,o] = sum_{i,k} phi(t[b,i]-k) * y[i,o,k] + psi(t[b,i]-k) * H*m[i,o,k]
with phi(s) = r^2(3-2r), psi(s) = s*r^2, r = relu(1-|s|);  m = pchip slopes
(functions of the parameter y only, precomputed host-side).  The device
computes a dense (2*D_IN*K x B) weight matrix on-chip from x and contracts it
with the (2*D_IN*K x D_OUT) tables on the PE.

Sharding: contraction-parallel over d_in — core c owns i in [32c, 32c+32).
Host sums the 8 partial (D_OUT, B) outputs, transposes, adds bias.

Device pipeline per i-pair j (16 pairs of d_in rows):
  PE  : s = E_j^T @ [t; ones]  — one (c=33) matmul broadcasts the pair's two
        t rows across partition halves AND subtracts k (E carries a -k row).
  ACT : |s| ; r = relu(1-|s|) ; r2 = r^2
  DVE : phi'' = (r-1.5)*r2  [pairs with -2y table] ; psi = s*r2 [pairs w/ H*m]
  PE  : 4 accumulating matmuls, tables stationary (128x128), W moving (N=512),
        into two (o_half, B) PSUM accumulators (output kept transposed).

NOTE: this walrus build allows only ONE semaphore wait per instruction; a
post-scheduling pass splits extra waits onto same-engine NoOps.
"""
import sys
sys.path.insert(0, '/opt/trn_rl_repo')
import numpy as np

B, D_IN, D_OUT, K = 512, 256, 256, 64
D_MIN, D_MAX = -2.0, 2.0
H = (D_MAX - D_MIN) / (K - 1)
N_CORES = 8
I_PER = D_IN // N_CORES          # 32 d_in rows per core
NPAIR = I_PER // 2               # 16 i-pairs per core

# main-matmul dtype: "f32" (exact, 4 cyc/row), "f32r" (reduced mantissa),
# or "f16" (1 cyc/row, fast weight load, 10-bit mantissa)
MAIN_DT = "f16"
F32R_TRUNC_BITS = 10  # low mantissa bits zeroed on the f32r grid (empirical)

_CACHE = {}


def _pchip_hm(y):
    """H * pchip_slopes(y), float64 internally, mirroring reference._pchip_slopes."""
    y = y.astype(np.float64)
    delta = (y[..., 1:] - y[..., :-1]) / H
    d0, d1 = delta[..., :-1], delta[..., 1:]
    denom = d0 + d1
    small = np.abs(denom) < 1e-12
    hm = 2.0 * d0 * d1 / np.where(small, 1.0, denom)
    hm = np.where(small, 0.0, hm)
    m_inner = np.where(d0 * d1 > 0, hm, 0.0)
    m0 = (3.0 * delta[..., 0] - delta[..., 1]) / 2.0
    mN = (3.0 * delta[..., -1] - delta[..., -2]) / 2.0
    m0 = np.where(m0 * delta[..., 0] <= 0, 0.0, m0)
    mN = np.where(mN * delta[..., -1] <= 0, 0.0, mN)
    cond0 = (delta[..., 0] * delta[..., 1] < 0) & (np.abs(m0) > np.abs(3.0 * delta[..., 0]))
    m0 = np.where(cond0, 3.0 * delta[..., 0], m0)
    condN = (delta[..., -1] * delta[..., -2] < 0) & (np.abs(mN) > np.abs(3.0 * delta[..., -1]))
    mN = np.where(condN, 3.0 * delta[..., -1], mN)
    m = np.concatenate([m0[..., None], m_inner, mN[..., None]], axis=-1)
    return (H * m).astype(np.float32)


def _round_f32r(a):
    """Round fp32 onto the f32r grid (truncate low mantissa bits)."""
    if F32R_TRUNC_BITS == 0:
        return a
    mask = np.uint32(0xFFFFFFFF) << np.uint32(F32R_TRUNC_BITS)
    return (a.view(np.uint32) & mask).view(np.float32)


def _build_tables(y):
    """Per-core rhs tables, shape (N_CORES, 2*K, 2*NPAIR, D_OUT).

    Table column group (j, h): h=0 -> +2*y rows for pair j, h=1 -> H*m rows.
    Row layout within a group: 64 k-rows of i0 then 64 k-rows of i1.
    Device loads this as a (128, 2*NPAIR*D_OUT) tile (32KB/partition,
    fully contiguous rows for DMA efficiency).
    """
    hm = _pchip_hm(y)                                       # (d_in, d_out, K)
    y2 = (2.0 * y.astype(np.float64)).astype(np.float32)
    y2_t = np.ascontiguousarray(np.transpose(y2, (0, 2, 1)))  # (d_in, K, d_out)
    hm_t = np.ascontiguousarray(np.transpose(hm, (0, 2, 1)))
    tbl = np.empty((N_CORES, NPAIR, 2, 2, K, D_OUT), np.float32)
    for c in range(N_CORES):
        i0 = c * I_PER
        tbl[c, :, 0] = y2_t[i0:i0 + I_PER].reshape(NPAIR, 2, K, D_OUT)
        tbl[c, :, 1] = hm_t[i0:i0 + I_PER].reshape(NPAIR, 2, K, D_OUT)
    # (c, j, h, half, k, o) -> rows (half,k) x cols (j,h,o)
    tbl = tbl.transpose(0, 3, 4, 1, 2, 5).reshape(N_CORES, 2 * K, 2 * NPAIR * D_OUT)
    if MAIN_DT == "f32r":
        tbl = _round_f32r(tbl)
    elif MAIN_DT == "f16":
        tbl = tbl.astype(np.float16)
    return np.ascontiguousarray(tbl)


def _build_selector():
    """E (66, NPAIR*128) fp16: per pair j a (66,128) stationary block.
    Rows 0-31 select t_hi rows (1.0 where (p<64, c==2j) or (p>=64, c==2j+1)),
    rows 32-63 repeat the selector for the t_lo rows, row 64 is -(p mod 64)
    (pairs with the ones-row).  All entries are fp16-exact (ints <= 63)."""
    e = np.zeros((65, NPAIR * 128), np.float16)
    for j in range(NPAIR):
        e[2 * j, j * 128:j * 128 + 64] = 1.0
        e[2 * j + 1, j * 128 + 64:(j + 1) * 128] = 1.0
        e[32 + 2 * j, j * 128:j * 128 + 64] = 1.0
        e[32 + 2 * j + 1, j * 128 + 64:(j + 1) * 128] = 1.0
    e[64] = np.tile(-(np.arange(128, dtype=np.float16) % 64), NPAIR)
    return e


def _build_bass():
    import concourse.bass as bass
    import concourse.tile as tile
    from concourse import mybir

    F32 = mybir.dt.float32
    DT = {"f32": F32, "f32r": mybir.dt.float32r,
          "f16": mybir.dt.float16}[MAIN_DT]
    ACTF = mybir.ActivationFunctionType
    ALU = mybir.AluOpType
    TW = 2 * NPAIR * D_OUT            # 8192 table columns

    F16 = mybir.dt.float16
    nc = bass.Bass()
    xt_d = nc.dram_tensor("xt", [33, B], F32, kind="ExternalInput")
    tbl_d = nc.dram_tensor("tbl", [2 * K, TW], DT, kind="ExternalInput")
    e_d = nc.dram_tensor("sel", [65, NPAIR * 128], F16, kind="ExternalInput")
    tc_d = nc.dram_tensor("tcol", [33, 1], F32, kind="ExternalInput")
    n1_d = nc.dram_tensor("n1col", [128, 1], F32, kind="ExternalInput")
    out_d = nc.dram_tensor("out", [D_OUT, B], F32, kind="ExternalOutput")

    with tile.TileContext(nc) as tc:
        with tc.tile_pool(name="const", bufs=1) as cpool, \
             tc.tile_pool(name="wk", bufs=3) as wkpool, \
             tc.tile_pool(name="wt", bufs=3) as wtpool, \
             tc.tile_pool(name="res", bufs=1) as respool, \
             tc.tile_pool(name="pacc", bufs=1, space="PSUM") as paccpool, \
             tc.tile_pool(name="pbc", bufs=3, space="PSUM") as pbcpool:

            e_t = cpool.tile([65, NPAIR * 128], F16)
            tc_t = cpool.tile([33, 1], F32)
            n1_t = cpool.tile([128, 1], F32)
            xt_t = cpool.tile([33, B], F32)
            # pre-warm the PE so the HAM clock gate opens before real work
            warm_t = cpool.tile([128, B], F16)
            nc.gpsimd.memset(warm_t[:], 0.0)
            # smalls go through SWDGE so the big table loads can't queue
            # ahead of them on the HWDGE rings; xt/tc first (t-prep gate)
            nc.gpsimd.dma_start(xt_t[:], xt_d[:])
            nc.gpsimd.dma_start(tc_t[:], tc_d[:])
            nc.gpsimd.dma_start(n1_t[:], n1_d[:])
            nc.sync.dma_start(e_t[:], e_d[:])
            tbl_t = cpool.tile([2 * K, TW], DT)
            for p in range(8):
                w = TW // 8
                nc.sync.dma_start(tbl_t[:, p * w:(p + 1) * w],
                                  tbl_d[:, p * w:(p + 1) * w])

            wacc = pbcpool.tile([128, B], F32, tag="bc", name="wacc")
            for _ in range(10):
                nc.tensor.matmul(wacc[:], warm_t[:, :128], warm_t[:],
                                 start=True, stop=True)

            # t = clip(x/H - D_MIN/H, 0, K-1); row 32 becomes exactly 1.0
            t_t = cpool.tile([33, B], F32)
            nc.scalar.activation(t_t[:], xt_t[:], ACTF.Relu,
                                 bias=tc_t[:], scale=1.0 / H)
            nc.vector.tensor_scalar_min(t_t[:], t_t[:], float(K - 1))
            # split t into fp16 hi + lo halves of a (65, B) rhs; the c=65
            # broadcast matmul then reconstructs t to ~2^-22 while running
            # at fp16 speed (single pass, fast weight load).  Rows: 0-31 hi,
            # 32-63 lo, 64 ones (for the -k term).
            t2_t = cpool.tile([65, B], F16)
            nc.scalar.copy(t2_t[0:32, :], t_t[0:32, :])
            nc.scalar.copy(t2_t[64:65, :], t_t[32:33, :])
            nc.vector.tensor_sub(t2_t[32:64, :], t_t[0:32, :], t2_t[0:32, :])

            accT = [paccpool.tile([128, B], F32, tag=f"accT{q}", name=f"accT{q}")
                    for q in range(2)]

            for j in range(NPAIR):
                # s = E_j^T @ [t; 1]: broadcast pair's t rows minus k, (128, B)
                bacc = pbcpool.tile([128, B], F32, tag="bc")
                nc.tensor.matmul(bacc[:], e_t[:, j * 128:(j + 1) * 128], t2_t[:],
                                 start=True, stop=True)

                # z = min(|s|, 1)
                ab_t = wkpool.tile([128, B], F32, tag="ab")
                nc.scalar.activation(ab_t[:], bacc[:], ACTF.Abs)
                z_t = wkpool.tile([128, B], F32, tag="z")
                nc.vector.tensor_scalar_min(z_t[:], ab_t[:], 1.0)
                # r2 = (1-z)^2 = Square(z - 1)
                r2_t = wkpool.tile([128, B], F32, tag="r2")
                nc.scalar.activation(r2_t[:], z_t[:], ACTF.Square, bias=n1_t[:])
                # phi3 = (z+0.5)*r2 (= -phi_herm/2, pairs with +2y); on Pool
                phi_t = wtpool.tile([128, B], DT, tag="phi")
                nc.vector.scalar_tensor_tensor(phi_t[:], z_t[:], 0.5, r2_t[:],
                                               op0=ALU.add, op1=ALU.mult)
                psi_t = wtpool.tile([128, B], DT, tag="psi")
                nc.vector.tensor_mul(psi_t[:], bacc[:], r2_t[:])

                for h, w_t in ((1, psi_t), (0, phi_t)):
                    base = (j * 2 + h) * D_OUT
                    for q in range(2):
                        nc.tensor.matmul(
                            accT[q][:],
                            tbl_t[:, base + q * 128: base + (q + 1) * 128],
                            w_t[:],
                            start=(j == 0 and h == 1),
                            stop=(j == NPAIR - 1 and h == 0))

            for q in range(2):
                o_t = respool.tile([128, B], F32, tag=f"o{q}", name=f"o{q}")
                nc.scalar.copy(o_t[:], accT[q][:])
                nc.sync.dma_start(out_d[q * 128:(q + 1) * 128, :], o_t[:])

    return nc


def _split_multiwaits(nc):
    """Walrus in this build allows one semaphore wait per instruction.  Tile
    sometimes emits several; split the extras onto same-engine NoOps inserted
    immediately before the instruction (queue order preserves semantics)."""
    from concourse import mybir

    fix_id = 0
    for f in nc.m.functions:
        for blk in f.blocks:
            insts = blk.instructions
            out, changed = [], False
            for ins in insts:
                si = getattr(ins, "sync_info", None)
                waits = list(si.on_wait) if si and si.on_wait else []
                if len(waits) > 1:
                    for w in waits[:-1]:
                        nop = mybir.InstNoOp(name=f"I-fixw{fix_id}",
                                             engine=ins.engine)
                        fix_id += 1
                        nop.sync_info = mybir.SyncInfo(on_wait=[w], on_update=[])
                        out.append(nop)
                    ins.sync_info = mybir.SyncInfo(
                        on_wait=[waits[-1]], on_update=list(si.on_update))
                    changed = True
                out.append(ins)
            if changed:
                blk.instructions = out


def _get_compiled():
    if "nc" not in _CACHE:
        nc = _build_bass()
        _split_multiwaits(nc)
        _CACHE["nc"] = nc
    return _CACHE["nc"]


def _run(x, y, bias, trace=False):
    from concourse.bass_utils import run_bass_kernel_spmd

    x = np.asarray(x, np.float32)
    y = np.asarray(y, np.float32)
    bias = np.asarray(bias, np.float32)

    nc = _get_compiled()

    xs = np.ascontiguousarray(x.T)                     # (d_in, B)
    tbl = _build_tables(y)                             # (8, 128, 8192)
    e_np = _build_selector()
    tc_np = np.full((33, 1), -D_MIN / H, np.float32)
    tc_np[32, 0] = 0.0

    in_maps = []
    for c in range(N_CORES):
        xt = np.empty((33, B), np.float32)
        xt[:32] = xs[c * I_PER:(c + 1) * I_PER]
        xt[32] = H                                     # relu(H/H + 0) == 1.0
        in_maps.append({
            "xt": xt,
            "tbl": tbl[c],
            "sel": e_np,
            "tcol": tc_np,
            "n1col": np.full((128, 1), -1.0, np.float32),
        })
    res = run_bass_kernel_spmd(nc, in_maps, core_ids=list(range(N_CORES)),
                               trace=trace)
    partialT = np.stack([res.results[c]["out"] for c in range(N_CORES)])
    out = partialT.astype(np.float64).sum(axis=0).T + bias.astype(np.float64)
    return out.astype(np.float32), res


def kernel(x, y, bias):
    out, _ = _run(x, y, bias)
    return out

